# revision 1
# baseline (speedup 1.0000x reference)
"""Trainium2 Bass kernel for nn_Block_26628797235524 (Mamba-style cross-scan SSM block).

Sharding: batch B=8 -> one batch element per NeuronCore (8 cores, SPMD, no
collectives). Each core runs the full block for its batch element:
  in_proj -> conv(1x1x1)+silu -> dual-order selective scan (K=2, DIN=256,
  DST=16) -> combine -> layernorm -> gate -> out_proj.

Layout on chip: channel-major (128-partition d tiles, time on the free dim).
The sequential scan runs as `tensor_tensor_scan` (state = dA*state + dBu)
over 128 channels x 1024-step slabs, chained via the previous slab's last
column; 16 state dims (n) are handled as separate scan lanes.

kernel(**inputs) takes the FULL unsharded inputs and returns the FULL output.
"""

import os
import sys
from contextlib import ExitStack

import numpy as np

_RL = "/opt/trn_rl_repo"
if os.path.isdir(_RL) and _RL not in sys.path:
    sys.path.insert(0, _RL)

import concourse.bass as bass
import concourse.bacc as bacc
import concourse.tile as tile
from concourse import mybir
from concourse.bass_utils import run_bass_kernel_spmd

# Problem sizes (hardcoded per the task contract).
B, T, H, W, DIM = 8, 16, 16, 16, 128
DIN, DST, DTR, KG = 256, 16, 8, 2
L = T * H * W          # 4096
P = 128                # partitions
DH = DIN // P          # 2 d-half tiles per direction
LC = 1024              # scan slab length
NSLAB = L // LC        # 4
NCORES = 8

F32 = mybir.dt.float32
AF = mybir.ActivationFunctionType
ALU = mybir.AluOpType
MM_F = 512             # matmul free-dim chunk (one PSUM bank)
NMM = L // MM_F        # 8 chunks over L


def _declare_drams(nc):
    d = {}

    def inp(name, shape):
        d[name] = nc.dram_tensor(name, list(shape), F32, kind="ExternalInput")

    inp("xT", (P, L))                  # per-core batch slice, (DIM, L) channel-major
    inp("w_in", (P, 4 * P))            # in_proj_w.T  (128, 512)
    inp("conv_sc", (DH, P, 1))         # conv_w per d-half column
    inp("conv_bi", (DH, P, 1))         # conv_b
    inp("w_xproj", (KG, DH, P, 40))    # x_proj_w[k].T in 2 pi-chunks
    inp("w_dt", (KG, DTR, DIN))        # dt_w[k].T  (8, 256)
    inp("dt_bias", (KG, DH, P, 1))
    inp("a_mat", (KG, DH, P, DST))     # A = -exp(A_logs)
    inp("ds_vec", (KG, DH, P, 1))
    inp("lnw", (DH, P, 1))
    inp("lnb", (DH, P, 1))
    inp("w_out", (DH, P, P))           # out_proj_w.T in 2 pi-chunks
    inp("ident", (P, P))               # identity: PE copy/accumulate matmuls
    d["xs_dram"] = nc.dram_tensor("xs_dram", [KG, DH, P, L], F32)  # spilled xs
    d["bc_dram"] = nc.dram_tensor("bc_dram", [KG, 2, DST, L], F32)  # B/C rows for bcast
    d["z_dram"] = nc.dram_tensor("z_dram", [DH, P, L], F32)      # spilled silu(z)
    d["y0_dram"] = nc.dram_tensor("y0_dram", [DH, P, L], F32)    # spilled y_fwd (natural order)
    d["outT"] = nc.dram_tensor("outT", [P, L], F32, kind="ExternalOutput")
    return d


def _body(tc, d):
    nc = tc.nc
    with ExitStack() as ctx:
        const = ctx.enter_context(tc.tile_pool(name="const", bufs=1))

        # ---- constants ----
        w_in = const.tile([P, 4 * P], F32, tag="w_in", name="w_in")
        nc.sync.dma_start(w_in[:], d["w_in"][:])
        conv_sc = [const.tile([P, 1], F32, tag=f"csc{i}", name=f"csc{i}") for i in range(DH)]
        conv_bi = [const.tile([P, 1], F32, tag=f"cbi{i}", name=f"cbi{i}") for i in range(DH)]
        for i in range(DH):
            nc.sync.dma_start(conv_sc[i][:], d["conv_sc"][i])
            nc.sync.dma_start(conv_bi[i][:], d["conv_bi"][i])
        w_xproj = [[const.tile([P, 40], F32, tag=f"wxp{k}{i}", name=f"wxp{k}{i}") for i in range(DH)]
                   for k in range(KG)]
        w_dt = [const.tile([DTR, DIN], F32, tag=f"wdt{k}", name=f"wdt{k}") for k in range(KG)]
        dt_bias = [[const.tile([P, 1], F32, tag=f"dtb{k}{i}", name=f"dtb{k}{i}") for i in range(DH)]
                   for k in range(KG)]
        a_mat = [[const.tile([P, DST], F32, tag=f"am{k}{i}", name=f"am{k}{i}") for i in range(DH)]
                 for k in range(KG)]
        ds_vec = [[const.tile([P, 1], F32, tag=f"dsv{k}{i}", name=f"dsv{k}{i}") for i in range(DH)]
                  for k in range(KG)]
        for k in range(KG):
            nc.sync.dma_start(w_dt[k][:], d["w_dt"][k])
            for i in range(DH):
                nc.sync.dma_start(w_xproj[k][i][:], d["w_xproj"][k, i])
                nc.sync.dma_start(dt_bias[k][i][:], d["dt_bias"][k, i])
                nc.sync.dma_start(a_mat[k][i][:], d["a_mat"][k, i])
                nc.sync.dma_start(ds_vec[k][i][:], d["ds_vec"][k, i])
        lnw = [const.tile([P, 1], F32, tag=f"lnw{i}", name=f"lnw{i}") for i in range(DH)]
        lnb = [const.tile([P, 1], F32, tag=f"lnb{i}", name=f"lnb{i}") for i in range(DH)]
        w_out = [const.tile([P, P], F32, tag=f"wo{i}", name=f"wo{i}") for i in range(DH)]
        for i in range(DH):
            nc.sync.dma_start(lnw[i][:], d["lnw"][i])
            nc.sync.dma_start(lnb[i][:], d["lnb"][i])
            nc.sync.dma_start(w_out[i][:], d["w_out"][i])
        ones_col = const.tile([P, 1], F32, tag="ones_col", name="ones_col")
        nc.vector.memset(ones_col[:], 1.0)
        one_b = const.tile([P, 1], F32, tag="one_b", name="one_b")
        nc.vector.memset(one_b[:], 1.0)
        ones_row = const.tile([1, P], F32, tag="ones_row", name="ones_row")
        nc.vector.memset(ones_row[:], 1.0)
        ident = const.tile([P, P], F32, tag="ident", name="ident")
        nc.sync.dma_start(ident[:], d["ident"][:])

        y1pool = ctx.enter_context(tc.tile_pool(name="y1pool", bufs=1))

        # ========== Phase 1: in_proj + conv/silu; Phase 2: scan orderings ======
        with tc.tile_pool(name="p1", bufs=1) as p1pool, \
             tc.tile_pool(name="p1ps", bufs=4, space=bass.MemorySpace.PSUM) as p1ps:
            xT = p1pool.tile([P, L], F32, tag="xT", name="xT")
            nc.sync.dma_start(xT[:], d["xT"][:])
            xh_s = [p1pool.tile([P, L], F32, tag=f"xh{i}", name=f"xh{i}") for i in range(DH)]
            z_t = p1pool.tile([P, L], F32, tag="z_t", name="z_t")
            for po in range(4):
                for c in range(NMM):
                    pt = p1ps.tile([P, MM_F], F32, tag="mm", name="mm")
                    nc.tensor.matmul(
                        pt[:], w_in[:, po * P:(po + 1) * P],
                        xT[:, c * MM_F:(c + 1) * MM_F], start=True, stop=True)
                    if po < DH:  # xh rows: silu(v), v = xh*conv_w + conv_b
                        sg = p1pool.tile([P, MM_F], F32, tag="sg", name="sg", bufs=3)
                        nc.scalar.activation(sg[:], pt[:], AF.Sigmoid,
                                             bias=conv_bi[po][:],
                                             scale=conv_sc[po][:])
                        vv = p1pool.tile([P, MM_F], F32, tag="vv", name="vv", bufs=3)
                        nc.vector.tensor_scalar(vv[:], pt[:], conv_sc[po][:],
                                                conv_bi[po][:], ALU.mult, ALU.add)
                        nc.vector.tensor_tensor(
                            xh_s[po][:, c * MM_F:(c + 1) * MM_F], vv[:], sg[:],
                            ALU.mult)
                    else:        # z rows: silu(z) -> spill to DRAM
                        sg = p1pool.tile([P, MM_F], F32, tag="sg", name="sg", bufs=3)
                        nc.scalar.activation(sg[:], pt[:], AF.Sigmoid)
                        nc.vector.tensor_tensor(
                            z_t[:, c * MM_F:(c + 1) * MM_F], pt[:], sg[:], ALU.mult)
                if po >= DH:
                    nc.sync.dma_start(d["z_dram"][po - DH], z_t[:])

            # xs[k=0]: (h,w,t) reorder; xs[k=1]: reversed natural. Both -> DRAM.
            for i in range(DH):
                ord0 = p1pool.tile([P, L], F32, tag="ord0", name="ord0")
                src = xh_s[i][:].rearrange("p (t hw) -> p hw t", t=T, hw=H * W)
                dst = ord0[:].rearrange("p (hw t) -> p hw t", hw=H * W, t=T)
                nc.vector.tensor_copy(dst, src)
                nc.sync.dma_start(d["xs_dram"][0, i], ord0[:])
                rev = p1pool.tile([P, L], F32, tag="rev", name="rev")
                nc.vector.tensor_copy(rev[:], xh_s[i][:, ::-1])
                nc.sync.dma_start(d["xs_dram"][1, i], rev[:])

        # ================= Per-direction pipeline =================
        y1_tiles = []
        for k in range(KG):
            with tc.tile_pool(name=f"kp{k}", bufs=1) as kpool:
                delta = [kpool.tile([P, L], F32, tag=f"delta{i}", name=f"delta{i}")
                         for i in range(DH)]
                # ---- Phase 3: x_dbl (dts-in -> SBUF; B/C rows -> DRAM) ----
                with tc.tile_pool(name=f"kd{k}", bufs=1) as kdpool, \
                     tc.tile_pool(name=f"kps{k}", bufs=4,
                                  space=bass.MemorySpace.PSUM) as kps:
                    xs_d = [kdpool.tile([P, L], F32, tag=f"xsd{i}", name=f"xsd{i}")
                            for i in range(DH)]
                    for i in range(DH):
                        nc.sync.dma_start(xs_d[i][:], d["xs_dram"][k, i])
                    xdbl_d = kdpool.tile([DTR, L], F32, tag="xdbl_d", name="xdbl_d")
                    parts = [(0, DTR, None), (DTR, DST, 0), (DTR + DST, DST, 1)]
                    for c in range(NMM):
                        for row0, nrow, bc in parts:
                            pt = kps.tile([DST, MM_F], F32, tag="mmx", name="mmx")
                            nc.tensor.matmul(pt[:nrow, :],
                                             w_xproj[k][0][:, row0:row0 + nrow],
                                             xs_d[0][:, c * MM_F:(c + 1) * MM_F],
                                             start=True, stop=False)
                            nc.tensor.matmul(pt[:nrow, :],
                                             w_xproj[k][1][:, row0:row0 + nrow],
                                             xs_d[1][:, c * MM_F:(c + 1) * MM_F],
                                             start=False, stop=True)
                            if bc is None:
                                nc.scalar.activation(
                                    xdbl_d[:, c * MM_F:(c + 1) * MM_F],
                                    pt[:nrow, :], AF.Copy)
                            else:
                                bcs = kdpool.tile([DST, MM_F], F32, tag="bcs",
                                                  name="bcs", bufs=2)
                                nc.scalar.activation(bcs[:], pt[:nrow, :], AF.Copy)
                                nc.sync.dma_start(
                                    d["bc_dram"][k, bc, :,
                                                 c * MM_F:(c + 1) * MM_F],
                                    bcs[:])
                    for i in range(DH):
                        for c in range(NMM):
                            pt2 = kps.tile([P, MM_F], F32, tag="mmd", name="mmd")
                            nc.tensor.matmul(pt2[:], w_dt[k][:, i * P:(i + 1) * P],
                                             xdbl_d[:, c * MM_F:(c + 1) * MM_F],
                                             start=True, stop=True)
                            # softplus(dts + dt_b) = ln(1 + exp(dts + dt_b))
                            ed = kdpool.tile([P, MM_F], F32, tag="ed", name="ed",
                                             bufs=2)
                            nc.scalar.activation(ed[:], pt2[:], AF.Exp,
                                                 bias=dt_bias[k][i][:])
                            nc.scalar.activation(delta[i][:, c * MM_F:(c + 1) * MM_F],
                                                 ed[:], AF.Ln, bias=one_b[:])

                    # ---- Phase 4 prep (uses xs_d while still resident) ----
                    dU = [kpool.tile([P, L], F32, tag=f"dU{i}", name=f"dU{i}")
                          for i in range(DH)]
                    state = [kpool.tile([P, DST], F32, tag=f"st{i}",
                                        name=f"st{i}") for i in range(DH)]
                    if k == 1:
                        y_sb = [y1pool.tile([P, L], F32, tag=f"y1_{i}",
                                            name=f"y1_{i}") for i in range(DH)]
                        y1_tiles.extend(y_sb)
                    else:
                        y_sb = [kpool.tile([P, L], F32, tag=f"y0_{i}",
                                           name=f"y0_{i}") for i in range(DH)]
                    for i in range(DH):
                        nc.vector.tensor_tensor(dU[i][:], delta[i][:], xs_d[i][:],
                                                ALU.mult)
                        # y init: Ds * u (skip-connection); scan adds from PSUM
                        nc.vector.tensor_scalar_mul(y_sb[i][:], xs_d[i][:],
                                                    ds_vec[k][i][:])
                        nc.vector.memset(state[i][:], 0.0)

                # ---- Phase 4: selective scan (k -> s -> n -> dh) ----
                with tc.tile_pool(name=f"sc{k}", bufs=2) as work, \
                     tc.tile_pool(name=f"scps{k}", bufs=2,
                                  space=bass.MemorySpace.PSUM) as scps:
                    for s in range(NSLAB):
                        sl = slice(s * LC, (s + 1) * LC)
                        y_ps = [scps.tile([P, LC], F32, tag=f"yps{i}",
                                          name=f"yps{i}") for i in range(DH)]
                        for n in range(DST):
                            brep = work.tile([P, LC], F32, tag="brep", name="brep")
                            nc.sync.dma_start(
                                brep[:],
                                d["bc_dram"][k, 0, n:n + 1, sl].partition_broadcast(P))
                            crep = work.tile([P, LC], F32, tag="crep", name="crep")
                            nc.sync.dma_start(
                                crep[:],
                                d["bc_dram"][k, 1, n:n + 1, sl].partition_broadcast(P))
                            for i in range(DH):
                                dA = work.tile([P, LC], F32, tag=f"dA{i}",
                                               name=f"dA{i}")
                                nc.scalar.activation(dA[:], delta[i][:, sl], AF.Exp,
                                                     scale=a_mat[k][i][:, n:n + 1])
                                xin = work.tile([P, LC], F32, tag="xin",
                                                name="xin")
                                xin_eng = nc.gpsimd if n >= 9 else nc.vector
                                xin_eng.tensor_tensor(xin[:], dU[i][:, sl], brep[:],
                                                      ALU.mult)
                                h = work.tile([P, LC], F32, tag=f"h{i}",
                                              name=f"h{i}")
                                nc.vector.tensor_tensor_scan(
                                    h[:], dA[:], xin[:], state[i][:, n:n + 1],
                                    ALU.mult, ALU.add)
                                nc.vector.tensor_copy(state[i][:, n:n + 1],
                                                      h[:, LC - 1:LC])
                                tmp = work.tile([P, LC], F32, tag="tmp",
                                                name="tmp")
                                nc.gpsimd.tensor_tensor(tmp[:], crep[:], h[:],
                                                        ALU.mult)
                                for hb in range(LC // MM_F):
                                    ps_ = slice(hb * MM_F, (hb + 1) * MM_F)
                                    nc.tensor.matmul(y_ps[i][:, ps_], ident[:],
                                                     tmp[:, ps_],
                                                     start=(n == 0),
                                                     stop=(n == DST - 1))
                        for i in range(DH):
                            nc.vector.scalar_tensor_tensor(
                                y_sb[i][:, sl], y_ps[i][:], 1.0, y_sb[i][:, sl],
                                ALU.mult, ALU.add)
                if k == 0:
                    for i in range(DH):
                        nc.sync.dma_start(d["y0_dram"][i], y_sb[i][:])

        # ================= Phase 5-7: combine, LN, gate, out_proj =================
        with tc.tile_pool(name="fin", bufs=1) as fin:
            y1 = y1_tiles
            ysum = [fin.tile([P, L], F32, tag=f"ys{i}", name=f"ys{i}")
                    for i in range(DH)]
            for i in range(DH):
                y0n = fin.tile([P, L], F32, tag="y0n", name="y0n")
                nc.sync.dma_start(y0n[:], d["y0_dram"][i])
                # y = reorder(y_fwd) + flip(y_rvs), in (t, hw) natural order
                src0 = y0n[:].rearrange("p (hw t) -> p t hw", hw=H * W, t=T)
                src1 = y1[i][:, ::-1].rearrange("p (t hw) -> p t hw", t=T, hw=H * W)
                dst = ysum[i][:].rearrange("p (t hw) -> p t hw", t=T, hw=H * W)
                nc.vector.tensor_tensor(dst, src0, src1, ALU.add)

            # LN stats over DIN (partition reduce via PE ones-contraction)
            stat_mu = fin.tile([1, L], F32, tag="stat_mu", name="stat_mu")
            stat_b = fin.tile([1, L], F32, tag="stat_b", name="stat_b")
            stat_r = fin.tile([1, L], F32, tag="stat_r", name="stat_r")
            with tc.tile_pool(name="fps1", bufs=4,
                              space=bass.MemorySpace.PSUM) as fps1:
                for c in range(NMM):
                    cs = slice(c * MM_F, (c + 1) * MM_F)
                    pmu = fps1.tile([1, MM_F], F32, tag="pmu", name="pmu")
                    nc.tensor.matmul(pmu[:], ones_col[:], ysum[0][:, cs],
                                     start=True, stop=False)
                    nc.tensor.matmul(pmu[:], ones_col[:], ysum[1][:, cs],
                                     start=False, stop=True)
                    nc.scalar.activation(stat_mu[:, cs], pmu[:], AF.Copy)
                    psq = fps1.tile([1, MM_F], F32, tag="psq", name="psq")
                    for i in range(DH):
                        ysq = fin.tile([P, MM_F], F32, tag="ysq", name="ysq")
                        nc.scalar.activation(ysq[:], ysum[i][:, cs], AF.Square)
                        nc.tensor.matmul(psq[:], ones_col[:], ysq[:],
                                         start=(i == 0), stop=(i == DH - 1))
                    nc.scalar.activation(stat_b[:, cs], psq[:], AF.Copy)
            nc.vector.tensor_scalar_mul(stat_mu[:], stat_mu[:], 1.0 / DIN)
            nc.vector.tensor_tensor(stat_r[:], stat_mu[:], stat_mu[:], ALU.mult)
            nc.vector.scalar_tensor_tensor(stat_b[:], stat_b[:], 1.0 / DIN,
                                           stat_r[:], ALU.mult, ALU.subtract)
            eps = fin.tile([1, 1], F32, tag="eps", name="eps")
            nc.vector.memset(eps[:], 1e-5)
            nc.scalar.activation(stat_r[:], stat_b[:], AF.Sqrt, bias=eps[:])
            nc.vector.reciprocal(stat_b[:], stat_r[:])
            mu, rstd = stat_mu, stat_b

            # normalize + affine + gate + out_proj, chunked over L
            with tc.tile_pool(name="fch", bufs=2) as fch, \
                 tc.tile_pool(name="fps2", bufs=2,
                              space=bass.MemorySpace.PSUM) as fps2:
                for c in range(NMM):
                    cs = slice(c * MM_F, (c + 1) * MM_F)
                    murep = fps2.tile([P, MM_F], F32, tag="murep", name="murep")
                    nc.tensor.matmul(murep[:], ones_row[:], mu[:, cs],
                                     start=True, stop=True)
                    rrep = fps2.tile([P, MM_F], F32, tag="rrep", name="rrep")
                    nc.tensor.matmul(rrep[:], ones_row[:], rstd[:, cs],
                                     start=True, stop=True)
                    g = []
                    for i in range(DH):
                        yc = fch.tile([P, MM_F], F32, tag="yc", name="yc")
                        nc.vector.tensor_tensor(yc[:], ysum[i][:, cs], murep[:],
                                                ALU.subtract)
                        yn = fch.tile([P, MM_F], F32, tag="yn", name="yn")
                        nc.vector.tensor_tensor(yn[:], yc[:], rrep[:], ALU.mult)
                        ya = fch.tile([P, MM_F], F32, tag="ya", name="ya")
                        nc.scalar.activation(ya[:], yn[:], AF.Identity,
                                             bias=lnb[i][:], scale=lnw[i][:])
                        zc = fch.tile([P, MM_F], F32, tag=f"zc{i}", name=f"zc{i}")
                        nc.sync.dma_start(zc[:], d["z_dram"][i, :, cs])
                        gi = fch.tile([P, MM_F], F32, tag=f"g{i}", name=f"g{i}")
                        nc.vector.tensor_tensor(gi[:], ya[:], zc[:], ALU.mult)
                        g.append(gi)
                    po = fps2.tile([P, MM_F], F32, tag="pout", name="pout")
                    nc.tensor.matmul(po[:], w_out[0][:], g[0][:],
                                     start=True, stop=False)
                    nc.tensor.matmul(po[:], w_out[1][:], g[1][:],
                                     start=False, stop=True)
                    osb = fch.tile([P, MM_F], F32, tag="osb", name="osb")
                    nc.scalar.activation(osb[:], po[:], AF.Copy)
                    nc.sync.dma_start(d["outT"][:, cs], osb[:])


_CACHE = {}


def _get_program():
    if "nc" not in _CACHE:
        nc = bacc.Bacc("TRN2", target_bir_lowering=False, debug=False,
                       num_devices=NCORES)
        d = _declare_drams(nc)
        with tile.TileContext(nc) as tc:
            _body(tc, d)
        nc.compile()
        _CACHE["nc"] = nc
    return _CACHE["nc"]


def _host_weights(inputs):
    f = lambda a: np.ascontiguousarray(np.asarray(a, np.float32))
    in_proj_w = f(inputs["in_proj_w"])          # (512, 128)
    x_proj_w = f(inputs["x_proj_w"])            # (2, 40, 256)
    dt_w = f(inputs["dt_w"])                    # (2, 256, 8)
    dt_b = f(inputs["dt_b"])                    # (2, 256)
    A_logs = f(inputs["A_logs"])                # (512, 16)
    Ds = f(inputs["Ds"])                        # (512,)
    m = {
        "w_in": f(in_proj_w.T),                                     # (128, 512)
        "conv_sc": f(inputs["conv_w"]).reshape(DH, P, 1),
        "conv_bi": f(inputs["conv_b"]).reshape(DH, P, 1),
        "w_xproj": f(x_proj_w.transpose(0, 2, 1).reshape(KG, DH, P, 40)),
        "w_dt": f(dt_w.transpose(0, 2, 1)),                         # (2, 8, 256)
        "dt_bias": f(dt_b).reshape(KG, DH, P, 1),
        "a_mat": f(-np.exp(A_logs)).reshape(KG, DH, P, DST),
        "ds_vec": f(Ds).reshape(KG, DH, P, 1),
        "lnw": f(inputs["ln_w"]).reshape(DH, P, 1),
        "lnb": f(inputs["ln_b"]).reshape(DH, P, 1),
        "w_out": f(f(inputs["out_proj_w"]).T.reshape(DH, P, P)),
    }
    m["ident"] = np.eye(P, dtype=np.float32)
    return m


def kernel(**inputs):
    x = np.ascontiguousarray(np.asarray(inputs["x"], np.float32))   # (8,16,16,16,128)
    shared = _host_weights(inputs)
    nc = _get_program()
    in_maps = []
    for b in range(NCORES):
        m = dict(shared)
        m["xT"] = np.ascontiguousarray(x[b].reshape(L, DIM).T)
        in_maps.append(m)
    trace = bool(int(os.environ.get("BASS_PROFILE", "0")))
    res = run_bass_kernel_spmd(nc, in_maps, list(range(NCORES)), trace=trace)
    _CACHE["last_result"] = res
    outs = [r["outT"] for r in res.results]
    out = np.stack([o.T.reshape(T, H, W, DIM) for o in outs]).astype(np.float32)
    return out



# revision 8
# speedup vs baseline: 1.5375x; 1.5375x over previous
"""Trainium2 Bass kernel for nn_Block_26628797235524 (Mamba-style cross-scan SSM block).

Sharding: batch B=8 -> one batch element per NeuronCore (8 cores, SPMD, no
collectives). Each core runs the full block for its batch element:
  in_proj -> conv(1x1x1)+silu -> dual-order selective scan (K=2, DIN=256,
  DST=16) -> combine -> layernorm -> gate -> out_proj.

v2: fp16 hot path (DVE 2-byte fast modes, 1-cyc/row PE matmuls), fused
Silu/Softplus activations, Ds skip-connection folded into the PSUM
accumulation via a diagonal matmul, xs/z kept resident in SBUF (no DRAM
round-trips), fp16 B/C broadcasts. Scan runs as tensor_tensor_scan over
128-channel x 1024-step slabs chained via the last column; state is fp32
inside the scan instruction so fp16 operands only quantize the readout.

kernel(**inputs) takes the FULL unsharded inputs and returns the FULL output.
"""

import os
import sys
from contextlib import ExitStack

import numpy as np

_RL = "/opt/trn_rl_repo"
if os.path.isdir(_RL) and _RL not in sys.path:
    sys.path.insert(0, _RL)

import concourse.bass as bass
import concourse.bacc as bacc
import concourse.tile as tile
from concourse import mybir
from concourse.bass_utils import run_bass_kernel_spmd

# Problem sizes (hardcoded per the task contract).
B, T, H, W, DIM = 8, 16, 16, 16, 128
DIN, DST, DTR, KG = 256, 16, 8, 2
L = T * H * W          # 4096
P = 128                # partitions
DH = DIN // P          # 2 d-half tiles per direction
LC = 1024              # scan slab length
NQ = L // LC           # 4
NCORES = 8

F32 = mybir.dt.float32
F16 = mybir.dt.float16
AF = mybir.ActivationFunctionType
ALU = mybir.AluOpType
MM_F = 512             # matmul free-dim chunk (one PSUM bank)
NMM = L // MM_F        # 8 chunks over L

# Engine split knobs for the scan inner loop (tuned from traces).
TMP_ON_GPSIMD = lambda n, i: (n % 2) == 0   # half of C-mults on gpsimd


def _declare_drams(nc):
    d = {}

    def inp(name, shape, dt=F16):
        d[name] = nc.dram_tensor(name, list(shape), dt, kind="ExternalInput")

    inp("xT", (P, L))                       # per-core batch slice, channel-major
    inp("w_in", (P, 4 * P))                 # in_proj_w.T
    inp("conv_sc", (DH, P, 1), F32)
    inp("conv_bi", (DH, P, 1), F32)
    inp("w_xproj", (KG, DH, P, 40))         # x_proj_w[k].T in 2 pi-chunks
    inp("w_dt", (KG, DTR, DIN))             # dt_w[k].T
    inp("dt_bias", (KG, DH, P, 1), F32)
    inp("a_mat", (KG, DH, P, DST), F32)     # A = -exp(A_logs)
    inp("ds_diag", (KG, DH, P, P))          # diag(Ds) per (k, half)
    inp("lnw", (DH, P, 1), F32)
    inp("lnb", (DH, P, 1), F32)
    inp("w_out", (DH, P, P))                # out_proj_w.T in 2 pi-chunks
    inp("ident", (P, P))                    # identity: PE accumulate matmuls
    d["bc_dram"] = nc.dram_tensor("bc_dram", [KG, 2 * DST, L], F16)  # B/C rows
    d["outT"] = nc.dram_tensor("outT", [P, L], F32, kind="ExternalOutput")
    return d


def _body(tc, d):
    nc = tc.nc
    with ExitStack() as ctx:
        const = ctx.enter_context(tc.tile_pool(name="const", bufs=1))

        # ---- constants ----
        w_in = const.tile([P, 4 * P], F16, tag="w_in", name="w_in")
        nc.sync.dma_start(w_in[:], d["w_in"][:])
        conv_sc = [const.tile([P, 1], F32, tag=f"csc{i}", name=f"csc{i}") for i in range(DH)]
        conv_bi = [const.tile([P, 1], F32, tag=f"cbi{i}", name=f"cbi{i}") for i in range(DH)]
        for i in range(DH):
            nc.sync.dma_start(conv_sc[i][:], d["conv_sc"][i])
            nc.sync.dma_start(conv_bi[i][:], d["conv_bi"][i])
        w_xproj = [[const.tile([P, 40], F16, tag=f"wxp{k}{i}", name=f"wxp{k}{i}") for i in range(DH)]
                   for k in range(KG)]
        w_dt = [const.tile([DTR, DIN], F16, tag=f"wdt{k}", name=f"wdt{k}") for k in range(KG)]
        dt_bias = [[const.tile([P, 1], F32, tag=f"dtb{k}{i}", name=f"dtb{k}{i}") for i in range(DH)]
                   for k in range(KG)]
        a_mat = [[const.tile([P, DST], F32, tag=f"am{k}{i}", name=f"am{k}{i}") for i in range(DH)]
                 for k in range(KG)]
        ds_diag = [[const.tile([P, P], F16, tag=f"dsd{k}{i}", name=f"dsd{k}{i}") for i in range(DH)]
                   for k in range(KG)]
        for k in range(KG):
            nc.sync.dma_start(w_dt[k][:], d["w_dt"][k])
            for i in range(DH):
                nc.sync.dma_start(w_xproj[k][i][:], d["w_xproj"][k, i])
                nc.sync.dma_start(dt_bias[k][i][:], d["dt_bias"][k, i])
                nc.sync.dma_start(a_mat[k][i][:], d["a_mat"][k, i])
                nc.sync.dma_start(ds_diag[k][i][:], d["ds_diag"][k, i])
        lnw = [const.tile([P, 1], F32, tag=f"lnw{i}", name=f"lnw{i}") for i in range(DH)]
        lnb = [const.tile([P, 1], F32, tag=f"lnb{i}", name=f"lnb{i}") for i in range(DH)]
        w_out = [const.tile([P, P], F16, tag=f"wo{i}", name=f"wo{i}") for i in range(DH)]
        for i in range(DH):
            nc.sync.dma_start(lnw[i][:], d["lnw"][i])
            nc.sync.dma_start(lnb[i][:], d["lnb"][i])
            nc.sync.dma_start(w_out[i][:], d["w_out"][i])
        # 1/DIN-scaled ones column: the LN mean/second-moment contraction.
        oneN_col = const.tile([P, 1], F16, tag="oneN_col", name="oneN_col")
        nc.vector.memset(oneN_col[:], 1.0 / DIN)
        ones_row = const.tile([1, P], F16, tag="ones_row", name="ones_row")
        nc.vector.memset(ones_row[:], 1.0)
        ident = const.tile([P, P], F16, tag="ident", name="ident")
        nc.sync.dma_start(ident[:], d["ident"][:])
        eps = const.tile([1, 1], F32, tag="eps", name="eps")
        nc.vector.memset(eps[:], 1e-5)

        # ---- persistent activations (fp16 [P, L] = 8KB/partition each) ----
        main = ctx.enter_context(tc.tile_pool(name="main", bufs=1))
        xs = [[main.tile([P, L], F16, tag=f"xs{k}{i}", name=f"xs{k}{i}")
               for i in range(DH)] for k in range(KG)]
        z_sb = [main.tile([P, L], F16, tag=f"z{i}", name=f"z{i}") for i in range(DH)]
        y_k = [[main.tile([P, L], F16, tag=f"y{k}{i}", name=f"y{k}{i}")
                for i in range(DH)] for k in range(KG)]

        # ========== Phase 1: in_proj -> conv+silu -> scan orderings ==========
        with tc.tile_pool(name="p1", bufs=1) as p1pool, \
             tc.tile_pool(name="p1ps", bufs=4, space=bass.MemorySpace.PSUM) as p1ps:
            xT = p1pool.tile([P, L], F16, tag="xT", name="xT")
            nc.sync.dma_start(xT[:], d["xT"][:])
            for po in range(4):
                for c in range(NMM):
                    cs = slice(c * MM_F, (c + 1) * MM_F)
                    pt = p1ps.tile([P, MM_F], F32, tag="mm", name="mm")
                    nc.tensor.matmul(pt[:], w_in[:, po * P:(po + 1) * P],
                                     xT[:, cs], start=True, stop=True)
                    if po < DH:
                        # xh = silu(xz*conv_w + conv_b), written twice:
                        # (h w t) order for k=0 and reversed (t h w) for k=1.
                        src3 = pt[:].rearrange("p (t hw) -> p t hw", t=2, hw=H * W)
                        dst3 = xs[0][po][:].rearrange(
                            "p (hw t) -> p t hw", hw=H * W, t=T)[:, 2 * c:2 * c + 2, :]
                        nc.scalar.activation(dst3, src3, AF.Silu,
                                             bias=conv_bi[po][:], scale=conv_sc[po][:])
                        rev = xs[1][po][:, ::-1]
                        nc.scalar.activation(rev[:, cs], pt[:], AF.Silu,
                                             bias=conv_bi[po][:], scale=conv_sc[po][:])
                    else:
                        nc.scalar.activation(z_sb[po - DH][:, cs], pt[:], AF.Silu)

        # ================= Per-direction pipeline =================
        for k in range(KG):
            with tc.tile_pool(name=f"kp{k}", bufs=1) as kpool:
                delta = [kpool.tile([P, L], F16, tag=f"delta{i}", name=f"delta{i}")
                         for i in range(DH)]
                dU = [kpool.tile([P, L], F16, tag=f"dU{i}", name=f"dU{i}")
                      for i in range(DH)]
                # ---- Phase 3: x_dbl -> (dts, B, C); delta = softplus ----
                with tc.tile_pool(name=f"kd{k}", bufs=1) as kdpool, \
                     tc.tile_pool(name=f"kps{k}", bufs=2,
                                  space=bass.MemorySpace.PSUM) as kps:
                    xdbl = kdpool.tile([DTR, L], F16, tag="xdbl", name="xdbl")
                    for c in range(NMM):
                        cs = slice(c * MM_F, (c + 1) * MM_F)
                        ptA = kps.tile([DTR, MM_F], F32, tag="mmA", name="mmA")
                        nc.tensor.matmul(ptA[:], w_xproj[k][0][:, 0:DTR],
                                         xs[k][0][:, cs], start=True, stop=False)
                        nc.tensor.matmul(ptA[:], w_xproj[k][1][:, 0:DTR],
                                         xs[k][1][:, cs], start=False, stop=True)
                        nc.scalar.activation(xdbl[:, cs], ptA[:], AF.Copy)
                        ptB = kps.tile([2 * DST, MM_F], F32, tag="mmB", name="mmB")
                        nc.tensor.matmul(ptB[:], w_xproj[k][0][:, DTR:40],
                                         xs[k][0][:, cs], start=True, stop=False)
                        nc.tensor.matmul(ptB[:], w_xproj[k][1][:, DTR:40],
                                         xs[k][1][:, cs], start=False, stop=True)
                        bcs = kdpool.tile([2 * DST, MM_F], F16, tag="bcs",
                                          name="bcs", bufs=3)
                        nc.scalar.activation(bcs[:], ptB[:], AF.Copy)
                        nc.sync.dma_start(d["bc_dram"][k, :, cs], bcs[:])
                    for i in range(DH):
                        for c in range(NMM):
                            cs = slice(c * MM_F, (c + 1) * MM_F)
                            pt2 = kps.tile([P, MM_F], F32, tag="mmd", name="mmd")
                            nc.tensor.matmul(pt2[:], w_dt[k][:, i * P:(i + 1) * P],
                                             xdbl[:, cs], start=True, stop=True)
                            # softplus(x + b) = ln(1 + exp(x + b))
                            ed = kdpool.tile([P, MM_F], F16, tag="ed",
                                             name="ed", bufs=3)
                            nc.scalar.activation(ed[:], pt2[:], AF.Exp,
                                                 bias=dt_bias[k][i][:])
                            nc.scalar.activation(delta[i][:, cs], ed[:],
                                                 AF.Ln, bias=1.0)
                    for i in range(DH):
                        nc.vector.tensor_tensor(dU[i][:], delta[i][:], xs[k][i][:],
                                                ALU.mult)

                # ---- Phase 4: selective scan (q -> n -> i) ----
                states = [kpool.tile([P, DST], F16, tag=f"st{i}", name=f"st{i}")
                          for i in range(DH)]
                with tc.tile_pool(name=f"sc{k}", bufs=2) as work, \
                     tc.tile_pool(name=f"bc{k}", bufs=3) as bcp, \
                     tc.tile_pool(name=f"scps{k}", bufs=1,
                                  space=bass.MemorySpace.PSUM) as scps:
                    for q in range(NQ):
                        sl = slice(q * LC, (q + 1) * LC)
                        y_ps = [scps.tile([P, LC], F32, tag=f"yps{i}",
                                          name=f"yps{i}") for i in range(DH)]
                        for i in range(DH):
                            for hb in range(LC // MM_F):
                                ps_ = slice(hb * MM_F, (hb + 1) * MM_F)
                                gs = slice(q * LC + hb * MM_F,
                                           q * LC + (hb + 1) * MM_F)
                                nc.tensor.matmul(y_ps[i][:, ps_], ds_diag[k][i][:],
                                                 xs[k][i][:, gs],
                                                 start=True, stop=False)
                        for n in range(DST):
                            brep = bcp.tile([P, LC], F16, tag="brep", name="brep")
                            nc.sync.dma_start(
                                brep[:],
                                d["bc_dram"][k, n:n + 1, sl].partition_broadcast(P))
                            crep = bcp.tile([P, LC], F16, tag="crep", name="crep")
                            nc.sync.dma_start(
                                crep[:],
                                d["bc_dram"][k, DST + n:DST + n + 1,
                                             sl].partition_broadcast(P))
                            for i in range(DH):
                                dA = work.tile([P, LC], F16, tag=f"dA{i}",
                                               name=f"dA{i}")
                                nc.scalar.activation(dA[:], delta[i][:, sl], AF.Exp,
                                                     scale=a_mat[k][i][:, n:n + 1])
                                xin = work.tile([P, LC], F16, tag="xin", name="xin")
                                nc.vector.tensor_tensor(xin[:], dU[i][:, sl],
                                                        brep[:], ALU.mult)
                                h = work.tile([P, LC], F16, tag=f"h{i}",
                                              name=f"h{i}")
                                init = 0.0 if q == 0 else states[i][:, n:n + 1]
                                nc.vector.tensor_tensor_scan(
                                    h[:], dA[:], xin[:], init, ALU.mult, ALU.add)
                                if q < NQ - 1:
                                    nc.vector.tensor_copy(states[i][:, n:n + 1],
                                                          h[:, LC - 1:LC])
                                tmp = work.tile([P, LC], F16, tag="tmp", name="tmp")
                                eng = nc.gpsimd if TMP_ON_GPSIMD(n, i) else nc.vector
                                eng.tensor_tensor(tmp[:], crep[:], h[:], ALU.mult)
                                for hb in range(LC // MM_F):
                                    ps_ = slice(hb * MM_F, (hb + 1) * MM_F)
                                    nc.tensor.matmul(y_ps[i][:, ps_], ident[:],
                                                     tmp[:, ps_],
                                                     start=False,
                                                     stop=(n == DST - 1))
                        for i in range(DH):
                            nc.scalar.activation(y_k[k][i][:, sl], y_ps[i][:],
                                                 AF.Copy)

        # ================= Phase 5-7: combine, LN, gate, out_proj =================
        with tc.tile_pool(name="fin", bufs=1) as fin:
            ysum = [fin.tile([P, L], F16, tag=f"ys{i}", name=f"ys{i}")
                    for i in range(DH)]
            for i in range(DH):
                # y = reorder(y_fwd) + flip(y_rvs), in (t, hw) natural order
                src0 = y_k[0][i][:].rearrange("p (hw t) -> p t hw", hw=H * W, t=T)
                src1 = y_k[1][i][:, ::-1].rearrange("p (t hw) -> p t hw", t=T, hw=H * W)
                dst = ysum[i][:].rearrange("p (t hw) -> p t hw", t=T, hw=H * W)
                nc.vector.tensor_tensor(dst, src0, src1, ALU.add)

            # LN stats over DIN (partition reduce via PE 1/DIN-ones contraction)
            mu16 = fin.tile([1, L], F16, tag="mu16", name="mu16")
            m2_16 = fin.tile([1, L], F16, tag="m2_16", name="m2_16")
            sq16 = fin.tile([1, L], F16, tag="sq16", name="sq16")
            rs16 = fin.tile([1, L], F16, tag="rs16", name="rs16")
            with tc.tile_pool(name="fps1", bufs=4,
                              space=bass.MemorySpace.PSUM) as fps1:
                for c in range(NMM):
                    cs = slice(c * MM_F, (c + 1) * MM_F)
                    pmu = fps1.tile([1, MM_F], F32, tag="pmu", name="pmu")
                    nc.tensor.matmul(pmu[:], oneN_col[:], ysum[0][:, cs],
                                     start=True, stop=False)
                    nc.tensor.matmul(pmu[:], oneN_col[:], ysum[1][:, cs],
                                     start=False, stop=True)
                    nc.scalar.activation(mu16[:, cs], pmu[:], AF.Copy)
                    psq = fps1.tile([1, MM_F], F32, tag="psq", name="psq")
                    for i in range(DH):
                        ysq = fin.tile([P, MM_F], F16, tag="ysq", name="ysq",
                                       bufs=2)
                        nc.scalar.activation(ysq[:], ysum[i][:, cs], AF.Square)
                        nc.tensor.matmul(psq[:], oneN_col[:], ysq[:],
                                         start=(i == 0), stop=(i == DH - 1))
                    nc.scalar.activation(m2_16[:, cs], psq[:], AF.Copy)
            # var = E[y^2] - mu^2 ; rstd = exp(-0.5*ln(var + eps))
            nc.vector.tensor_tensor(sq16[:], mu16[:], mu16[:], ALU.mult)
            nc.vector.tensor_tensor(m2_16[:], m2_16[:], sq16[:], ALU.subtract)
            nc.scalar.activation(sq16[:], m2_16[:], AF.Ln, bias=eps[:1, :])
            nc.scalar.activation(rs16[:], sq16[:], AF.Exp, scale=-0.5)

            # normalize + affine + gate + out_proj, chunked over L
            with tc.tile_pool(name="fch", bufs=2) as fch, \
                 tc.tile_pool(name="fps2", bufs=2,
                              space=bass.MemorySpace.PSUM) as fps2:
                for c in range(NMM):
                    cs = slice(c * MM_F, (c + 1) * MM_F)
                    pm = fps2.tile([P, MM_F], F32, tag="pm", name="pm")
                    nc.tensor.matmul(pm[:], ones_row[:], mu16[:, cs],
                                     start=True, stop=True)
                    mrep = fch.tile([P, MM_F], F16, tag="mrep", name="mrep")
                    nc.scalar.activation(mrep[:], pm[:], AF.Copy)
                    pr = fps2.tile([P, MM_F], F32, tag="pr", name="pr")
                    nc.tensor.matmul(pr[:], ones_row[:], rs16[:, cs],
                                     start=True, stop=True)
                    rrep = fch.tile([P, MM_F], F16, tag="rrep", name="rrep")
                    nc.scalar.activation(rrep[:], pr[:], AF.Copy)
                    g = []
                    for i in range(DH):
                        yc = fch.tile([P, MM_F], F16, tag="yc", name="yc")
                        nc.vector.tensor_tensor(yc[:], ysum[i][:, cs], mrep[:],
                                                ALU.subtract)
                        yn = fch.tile([P, MM_F], F16, tag="yn", name="yn")
                        nc.vector.tensor_tensor(yn[:], yc[:], rrep[:], ALU.mult)
                        ya = fch.tile([P, MM_F], F16, tag="ya", name="ya")
                        nc.scalar.activation(ya[:], yn[:], AF.Identity,
                                             bias=lnb[i][:], scale=lnw[i][:])
                        gi = fch.tile([P, MM_F], F16, tag=f"g{i}", name=f"g{i}")
                        nc.vector.tensor_tensor(gi[:], ya[:], z_sb[i][:, cs],
                                                ALU.mult)
                        g.append(gi)
                    po = fps2.tile([P, MM_F], F32, tag="pout", name="pout")
                    nc.tensor.matmul(po[:], w_out[0][:], g[0][:],
                                     start=True, stop=False)
                    nc.tensor.matmul(po[:], w_out[1][:], g[1][:],
                                     start=False, stop=True)
                    osb = fch.tile([P, MM_F], F32, tag="osb", name="osb")
                    nc.scalar.activation(osb[:], po[:], AF.Copy)
                    nc.sync.dma_start(d["outT"][:, cs], osb[:])


_CACHE = {}


def _get_program():
    if "nc" not in _CACHE:
        nc = bacc.Bacc("TRN2", target_bir_lowering=False, debug=False,
                       num_devices=NCORES)
        d = _declare_drams(nc)
        with tile.TileContext(nc) as tc:
            _body(tc, d)
        nc.compile()
        _CACHE["nc"] = nc
    return _CACHE["nc"]


def _host_weights(inputs):
    f32 = lambda a: np.ascontiguousarray(np.asarray(a, np.float32))
    f16 = lambda a: np.ascontiguousarray(np.asarray(a, np.float32).astype(np.float16))
    in_proj_w = f32(inputs["in_proj_w"])        # (512, 128)
    x_proj_w = f32(inputs["x_proj_w"])          # (2, 40, 256)
    dt_w = f32(inputs["dt_w"])                  # (2, 256, 8)
    dt_b = f32(inputs["dt_b"])                  # (2, 256)
    A_logs = f32(inputs["A_logs"])              # (512, 16)
    Ds = f32(inputs["Ds"])                      # (512,)
    ds_diag = np.zeros((KG, DH, P, P), np.float16)
    dsr = Ds.reshape(KG, DH, P)
    for k in range(KG):
        for i in range(DH):
            np.fill_diagonal(ds_diag[k, i], dsr[k, i].astype(np.float16))
    m = {
        "w_in": f16(in_proj_w.T),                                   # (128, 512)
        "conv_sc": f32(inputs["conv_w"]).reshape(DH, P, 1),
        "conv_bi": f32(inputs["conv_b"]).reshape(DH, P, 1),
        "w_xproj": f16(x_proj_w.transpose(0, 2, 1).reshape(KG, DH, P, 40)),
        "w_dt": f16(dt_w.transpose(0, 2, 1)),                       # (2, 8, 256)
        "dt_bias": f32(dt_b).reshape(KG, DH, P, 1),
        "a_mat": f32(-np.exp(A_logs)).reshape(KG, DH, P, DST),
        "ds_diag": ds_diag,
        "lnw": f32(inputs["ln_w"]).reshape(DH, P, 1),
        "lnb": f32(inputs["ln_b"]).reshape(DH, P, 1),
        "w_out": f16(f32(inputs["out_proj_w"]).T.reshape(DH, P, P)),
        "ident": np.eye(P, dtype=np.float16),
    }
    return m


def kernel(**inputs):
    x = np.ascontiguousarray(np.asarray(inputs["x"], np.float32))   # (8,16,16,16,128)
    shared = _host_weights(inputs)
    nc = _get_program()
    in_maps = []
    for b in range(NCORES):
        m = dict(shared)
        m["xT"] = np.ascontiguousarray(x[b].reshape(L, DIM).T).astype(np.float16)
        in_maps.append(m)
    trace = bool(int(os.environ.get("BASS_PROFILE", "0")))
    res = run_bass_kernel_spmd(nc, in_maps, list(range(NCORES)), trace=trace)
    _CACHE["last_result"] = res
    outs = [r["outT"] for r in res.results]
    out = np.stack([o.T.reshape(T, H, W, DIM) for o in outs]).astype(np.float32)
    return out
